# revision 5
# baseline (speedup 1.0000x reference)
"""Causal self-attention Trainium2 kernel (8 NeuronCores, SPMD).

Problem (hardcoded): B=2, T=2048, C=1024, H=16 heads, d=64.
  qkv = x @ W_qkv ; per-head causal softmax attention ; out @ W_proj.

Sharding: core m (0..7) handles batch b = m//4 and head group g = m%4
(heads 4g..4g+3). Each core computes q/k/v for its 4 heads (256 of the
3072 W_qkv columns), full TxT causal attention for those heads, and a
partial projection y_m = att_m @ W_proj[256g:256g+256, :].  The host
sums the 4 partials per batch (row-split tensor parallel reduce).

Device kernel layout notes (scores kept TRANSPOSED: [key j, query i]):
  - x is fed pre-transposed per batch: xT [C, T] (fp16).
  - qkv^T computed as matmul(lhsT=W block, rhs=xT block): q^T/k^T land
    in [head-ch, T] layout, exactly what QK^T needs (contract over d).
  - v is computed in natural [T, ch] layout (lhsT=xT block, rhs=Wv) and
    stored with an extra ones column per head, so the AV matmul also
    yields the softmax denominators as output row 64 (M = d+1 = 65).
  - scores^T tiles [128 j, 512 i]: only j-blocks <= diagonal are
    computed (causal skip ~2x FLOPs); diagonal tiles are masked AFTER
    exp by zeroing with gpsimd.affine_select (keep iff j <= i).
  - softmax divide: reciprocal of the sums row, broadcast across 64
    partitions with a K=1 ones-matmul, multiplied in while copying out
    of PSUM. Head 1 of each pair is partition-shifted 0-63 -> 64-127
    with an SBUF->SBUF DMA so att^T tiles [128 = 2 heads x 64, T] feed
    the projection matmul directly as lhsT.
"""

import numpy as np

import concourse.bass as bass
import concourse.mybir as mybir
import concourse.tile as tile
from concourse import bacc

FP32 = mybir.dt.float32
FP16 = mybir.dt.float16
AF = mybir.ActivationFunctionType
ALU = mybir.AluOpType

B, T_FULL, C_FULL, H_FULL, D_HEAD = 2, 2048, 1024, 16, 64
N_CORES = 8


def build_nc(T=T_FULL, C=C_FULL, HD=4, D=D_HEAD, n_cores=N_CORES):
    """Build the per-core Bass program. HD = heads per core."""
    CD = HD * D              # device head channels (256)
    CB = C // 128            # contraction blocks over x/W channels
    ICH = 512                # query-chunk width
    NI = T // ICH
    S = ICH // 128           # j-blocks per query chunk on the diagonal
    TC = 512                 # token chunk in qkv phase
    NTC = T // TC
    NTB = T // 128           # 128-token blocks (= key blocks)
    NPAIR = HD // 2
    JQK = CD // 128          # q (and k) 128-wide column blocks
    assert JQK == NPAIR and T % ICH == 0 and C % 512 == 0
    softmax_scale = 1.0 / float(np.sqrt(D))

    nc = bacc.Bacc(
        "TRN2", target_bir_lowering=False, debug=False, num_devices=n_cores
    )
    xT = nc.dram_tensor("xT", [C, T], FP16, kind="ExternalInput").ap()
    wqkv = nc.dram_tensor("wqkv", [C, 3 * CD], FP16, kind="ExternalInput").ap()
    wp = nc.dram_tensor("wp", [CD, C], FP16, kind="ExternalInput").ap()
    y = nc.dram_tensor("y", [T, C], FP16, kind="ExternalOutput").ap()

    with tile.TileContext(nc) as tc:
        with (
            tc.tile_pool(name="consts", bufs=1) as consts,
            tc.tile_pool(name="xt", bufs=2 * CB) as xt_pool,
            tc.tile_pool(name="ew", bufs=4) as ew_pool,
            tc.tile_pool(name="small", bufs=4) as small_pool,
            tc.tile_pool(name="ysb", bufs=4) as ysb_pool,
            tc.tile_pool(name="psb", bufs=2, space="PSUM") as ps_big,
            tc.tile_pool(name="psm", bufs=4, space="PSUM") as ps_med,
        ):
            # ---- resident tensors ----
            w_sb = consts.tile([128, CB, 3 * CD], FP16)
            nc.sync.dma_start(w_sb, wqkv.rearrange("(po pi) f -> pi po f", pi=128))
            wp_sb = consts.tile([128, CD // 128, C], FP16)
            nc.sync.dma_start(wp_sb, wp.rearrange("(po pi) f -> pi po f", pi=128))
            qT = consts.tile([128, NPAIR, T], FP16)
            kT = consts.tile([128, NPAIR, T], FP16)
            vS = consts.tile([128, NTB, HD, D + 1], FP16)
            nc.vector.memset(vS[:, :, :, D : D + 1], 1.0)
            attT = consts.tile([128, NPAIR, T], FP16)
            ones_sb = consts.tile([65, 64], FP16)
            nc.vector.memset(ones_sb[64:65, :], 1.0)

            # ================= phase 1: qkv =================
            for t in range(NTC):
                xts = []
                for cb in range(CB):
                    xt_t = xt_pool.tile([128, TC], FP16, tag="xt")
                    nc.sync.dma_start(
                        xt_t, xT[128 * cb : 128 * (cb + 1), TC * t : TC * (t + 1)]
                    )
                    xts.append(xt_t)
                # q^T / k^T: one [128, 2*TC] psum holds both column blocks
                for qk in range(2):       # 0 -> q, 1 -> k
                    p2 = ps_big.tile([128, JQK * TC], FP32, tag="big")
                    for jb in range(JQK):
                        co = CD * qk + 128 * jb
                        for cb in range(CB):
                            nc.tensor.matmul(
                                p2[:, TC * jb : TC * (jb + 1)],
                                w_sb[:, cb, co : co + 128],
                                xts[cb],
                                start=(cb == 0),
                                stop=(cb == CB - 1),
                            )
                    dst = qT if qk == 0 else kT
                    nc.vector.tensor_copy(
                        out=dst[:, :, TC * t : TC * (t + 1)],
                        in_=p2.rearrange("p (j f) -> p j f", j=JQK),
                    )
                # v in natural layout, one 128-token block at a time
                for tb in range(TC // 128):
                    tb_g = t * (TC // 128) + tb
                    pv = ps_med.tile([128, CD], FP32, tag="med")
                    for cb in range(CB):
                        nc.tensor.matmul(
                            pv,
                            xts[cb][:, 128 * tb : 128 * (tb + 1)],
                            w_sb[:, cb, 2 * CD : 3 * CD],
                            start=(cb == 0),
                            stop=(cb == CB - 1),
                        )
                    nc.vector.tensor_copy(
                        out=vS[:, tb_g, :, 0:D],
                        in_=pv.rearrange("p (h d) -> p h d", d=D),
                    )

            # ================= phase 2: attention =================
            for p in range(NPAIR):
                for ic in range(NI):
                    J = S * (ic + 1)      # kept key blocks for this chunk
                    n_grp = (J + 1) // 2
                    av = [
                        ps_med.tile([D + 1, ICH], FP32, tag="med", name=f"av{h2}")
                        for h2 in range(2)
                    ]

                    def qk_group(g, p=p, ic=ic, J=J):
                        """scores^T + exp + causal mask for 2 j-blocks,
                        both heads of the pair. Returns ew tiles."""
                        jbs = [jb for jb in (2 * g, 2 * g + 1) if jb < J]
                        ews = []
                        for h2 in range(2):
                            po = 64 * h2
                            sc = ps_big.tile(
                                [128, len(jbs) * ICH], FP32, tag="big"
                            )
                            for i_s, jb in enumerate(jbs):
                                nc.tensor.matmul(
                                    sc[:, ICH * i_s : ICH * (i_s + 1)],
                                    kT[po : po + 64, p, 128 * jb : 128 * (jb + 1)],
                                    qT[po : po + 64, p, ICH * ic : ICH * (ic + 1)],
                                    start=True,
                                    stop=True,
                                    tile_position=(po, 0),
                                )
                            ew = ew_pool.tile(
                                [128, len(jbs), ICH], FP16, tag="ew"
                            )
                            nc.scalar.activation(
                                ew.rearrange("p a b -> p (a b)"),
                                sc,
                                AF.Exp,
                                scale=softmax_scale,
                            )
                            for i_s, jb in enumerate(jbs):
                                s = jb - S * ic
                                if s >= 0:   # diagonal: zero where j > i
                                    nc.gpsimd.affine_select(
                                        out=ew[:, i_s],
                                        in_=ew[:, i_s],
                                        compare_op=ALU.is_ge,
                                        fill=0.0,
                                        base=-128 * s,
                                        pattern=[[1, ICH]],
                                        channel_multiplier=-1,
                                    )
                            ews.append((ew, jbs))
                        return ews

                    def av_group(ews, p=p, J=J):
                        for h2 in range(2):
                            ew, jbs = ews[h2]
                            for i_s, jb in enumerate(jbs):
                                nc.tensor.matmul(
                                    av[h2],
                                    vS[:, jb, 2 * p + h2, :],
                                    ew[:, i_s],
                                    start=(jb == 0),
                                    stop=(jb == J - 1),
                                )

                    # 1-group software pipeline: QK(g+1) before AV(g)
                    prev = qk_group(0)
                    for g in range(1, n_grp):
                        cur = qk_group(g)
                        av_group(prev)
                        prev = cur
                    av_group(prev)

                    # softmax denominator, divide, store att^T
                    for h2 in range(2):
                        recip = small_pool.tile([65, ICH], FP16, tag="recip")
                        with nc.allow_low_precision("softmax recip fp16"):
                            nc.vector.reciprocal(
                                recip[64:65, :], av[h2][D : D + 1, :]
                            )
                        bc = ps_med.tile([64, ICH], FP32, tag="med")
                        nc.tensor.matmul(
                            bc,
                            ones_sb[64:65, :],
                            recip[64:65, :],
                            start=True,
                            stop=True,
                            tile_position=(64, 0),
                        )
                        bc_sb = small_pool.tile([64, ICH], FP16, tag="bc")
                        nc.vector.tensor_copy(out=bc_sb, in_=bc)
                        if h2 == 0:
                            nc.vector.tensor_tensor(
                                attT[0:64, p, ICH * ic : ICH * (ic + 1)],
                                av[h2][0:D, :],
                                bc_sb,
                                ALU.mult,
                            )
                        else:
                            tmp = small_pool.tile([64, ICH], FP16, tag="tmp")
                            nc.vector.tensor_tensor(
                                tmp, av[h2][0:D, :], bc_sb, ALU.mult
                            )
                            # partition shift 0-63 -> 64-127 via DMA
                            nc.sync.dma_start(
                                attT[64:128, p, ICH * ic : ICH * (ic + 1)], tmp
                            )

            # ================= phase 3: projection =================
            for tb in range(NTB):
                for nck in range(C // 512):
                    py = ps_med.tile([128, 512], FP32, tag="med")
                    for p in range(NPAIR):
                        nc.tensor.matmul(
                            py,
                            attT[:, p, 128 * tb : 128 * (tb + 1)],
                            wp_sb[:, p, 512 * nck : 512 * (nck + 1)],
                            start=(p == 0),
                            stop=(p == NPAIR - 1),
                        )
                    y_sb = ysb_pool.tile([128, 512], FP16, tag="ysb")
                    nc.vector.tensor_copy(out=y_sb, in_=py)
                    nc.sync.dma_start(
                        y[128 * tb : 128 * (tb + 1), 512 * nck : 512 * (nck + 1)],
                        y_sb,
                    )
    nc.compile()
    return nc


def make_in_maps(x, W_qkv, W_proj, n_cores=N_CORES):
    """Host-side sharding: per-core fp16 inputs."""
    Bv, T, C = x.shape
    groups = n_cores // Bv
    CD = C // groups
    in_maps = []
    xT_b = [np.ascontiguousarray(x[b].T).astype(np.float16) for b in range(Bv)]
    for m in range(n_cores):
        b, g = m // groups, m % groups
        cols = slice(CD * g, CD * (g + 1))
        wqkv_dev = np.ascontiguousarray(
            np.concatenate(
                [W_qkv[:, 0:C][:, cols], W_qkv[:, C : 2 * C][:, cols],
                 W_qkv[:, 2 * C : 3 * C][:, cols]],
                axis=1,
            ).astype(np.float16)
        )
        wp_dev = np.ascontiguousarray(W_proj[cols, :]).astype(np.float16)
        in_maps.append({"xT": xT_b[b], "wqkv": wqkv_dev, "wp": wp_dev})
    return in_maps


_cache = {}


def _get_nc():
    if "nc" not in _cache:
        _cache["nc"] = build_nc()
    return _cache["nc"]


def run(x, W_qkv, W_proj, trace=False):
    """Run on hardware; returns (y_full, BassKernelResults)."""
    from concourse.bass_utils import run_bass_kernel_spmd

    nc = _get_nc()
    in_maps = make_in_maps(x, W_qkv, W_proj)
    res = run_bass_kernel_spmd(
        nc, in_maps, core_ids=list(range(N_CORES)), trace=trace
    )
    Bv, T, C = x.shape
    groups = N_CORES // Bv
    y_full = np.zeros((Bv, T, C), dtype=np.float32)
    for m in range(N_CORES):
        y_full[m // groups] += res.results[m]["y"].astype(np.float32)
    return y_full, res


def kernel(x, W_qkv, W_proj):
    y, _ = run(
        np.asarray(x, dtype=np.float32),
        np.asarray(W_qkv, dtype=np.float32),
        np.asarray(W_proj, dtype=np.float32),
    )
    return y


# revision 13
# speedup vs baseline: 1.0165x; 1.0165x over previous
"""Causal self-attention Trainium2 kernel (8 NeuronCores, SPMD).

Problem (hardcoded): B=2, T=2048, C=1024, H=16 heads, d=64.
  qkv = x @ W_qkv ; per-head causal softmax attention ; out @ W_proj.

Sharding: core m (0..7) handles batch b = m//4 and head group g = m%4
(heads 4g..4g+3). Each core computes q/k/v for its 4 heads (256 of the
3072 W_qkv columns), full TxT causal attention for those heads, and a
partial projection y_m = att_m @ W_proj[256g:256g+256, :].  The host
sums the 4 partials per batch (row-split tensor parallel reduce).

Device kernel layout notes (scores kept TRANSPOSED: [key j, query i]):
  - x is fed pre-transposed per batch: xT [C, T] (fp16).
  - qkv^T computed as matmul(lhsT=W block, rhs=xT block): q^T/k^T land
    in [head-ch, T] layout, exactly what QK^T needs (contract over d).
  - v is computed in natural [T, ch] layout (lhsT=xT block, rhs=Wv) and
    stored with an extra ones column per head, so the AV matmul also
    yields the softmax denominators as output row 64 (M = d+1 = 65).
  - scores^T tiles [128 j, 512 i]: only j-blocks <= diagonal are
    computed (causal skip ~2x FLOPs); diagonal tiles are masked AFTER
    exp by zeroing with gpsimd.affine_select (keep iff j <= i).
  - softmax divide: reciprocal of the sums row, broadcast across 64
    partitions with a K=1 ones-matmul, multiplied in while copying out
    of PSUM. Head 1 of each pair is partition-shifted 0-63 -> 64-127
    with an SBUF->SBUF DMA so att^T tiles [128 = 2 heads x 64, T] feed
    the projection matmul directly as lhsT.
"""

import numpy as np

import concourse.bass as bass
import concourse.mybir as mybir
import concourse.tile as tile
from concourse import bacc

FP32 = mybir.dt.float32
FP16 = mybir.dt.float16
AF = mybir.ActivationFunctionType
ALU = mybir.AluOpType

B, T_FULL, C_FULL, H_FULL, D_HEAD = 2, 2048, 1024, 16, 64
N_CORES = 8


def build_nc(T=T_FULL, C=C_FULL, HD=4, D=D_HEAD, n_cores=N_CORES):
    """Build the per-core Bass program. HD = heads per core."""
    CD = HD * D              # device head channels (256)
    CB = C // 128            # contraction blocks over x/W channels
    ICH = 512                # query-chunk width
    NI = T // ICH
    S = ICH // 128           # j-blocks per query chunk on the diagonal
    TC = 512                 # token chunk in qkv phase
    NTC = T // TC
    NTB = T // 128           # 128-token blocks (= key blocks)
    NPAIR = HD // 2
    JQK = CD // 128          # q (and k) 128-wide column blocks
    assert JQK == NPAIR and T % ICH == 0 and C % 512 == 0
    softmax_scale = 1.0 / float(np.sqrt(D))

    nc = bacc.Bacc(
        "TRN2", target_bir_lowering=False, debug=False, num_devices=n_cores
    )
    xT = nc.dram_tensor("xT", [C, T], FP16, kind="ExternalInput").ap()
    wqkv = nc.dram_tensor("wqkv", [C, 3 * CD], FP16, kind="ExternalInput").ap()
    wp = nc.dram_tensor("wp", [CD, C], FP16, kind="ExternalInput").ap()
    y = nc.dram_tensor("y", [T, C], FP16, kind="ExternalOutput").ap()

    with tile.TileContext(nc) as tc:
        with (
            tc.tile_pool(name="consts", bufs=1) as consts,
            tc.tile_pool(name="xt", bufs=2 * CB) as xt_pool,
            tc.tile_pool(name="ew", bufs=4) as ew_pool,
            tc.tile_pool(name="small", bufs=4) as small_pool,
            tc.tile_pool(name="ysb", bufs=4) as ysb_pool,
            tc.tile_pool(name="psb", bufs=2, space="PSUM") as ps_big,
            tc.tile_pool(name="psm", bufs=4, space="PSUM") as ps_med,
        ):
            # ---- resident tensors ----
            w_sb = consts.tile([128, CB, 3 * CD], FP16)
            nc.sync.dma_start(w_sb, wqkv.rearrange("(po pi) f -> pi po f", pi=128))
            wp_sb = consts.tile([128, CD // 128, C], FP16)
            nc.sync.dma_start(wp_sb, wp.rearrange("(po pi) f -> pi po f", pi=128))
            qT = consts.tile([128, NPAIR, T], FP16)
            kT = consts.tile([128, NPAIR, T], FP16)
            vS = consts.tile([128, NTB, HD, D + 1], FP16)
            nc.vector.memset(vS[:, :, :, D : D + 1], 1.0)
            attT = consts.tile([128, NPAIR, T], FP16)
            ones_sb = consts.tile([65, 64], FP16)
            nc.vector.memset(ones_sb[64:65, :], 1.0)

            # ================= phase 1: qkv =================
            for t in range(NTC):
                xts = []
                for cb in range(CB):
                    xt_t = xt_pool.tile([128, TC], FP16, tag="xt")
                    nc.sync.dma_start(
                        xt_t, xT[128 * cb : 128 * (cb + 1), TC * t : TC * (t + 1)]
                    )
                    xts.append(xt_t)
                # q^T / k^T: one [128, 2*TC] psum holds both column blocks
                for qk in range(2):       # 0 -> q, 1 -> k
                    p2 = ps_big.tile([128, JQK * TC], FP32, tag="big")
                    for jb in range(JQK):
                        co = CD * qk + 128 * jb
                        for cb in range(CB):
                            nc.tensor.matmul(
                                p2[:, TC * jb : TC * (jb + 1)],
                                w_sb[:, cb, co : co + 128],
                                xts[cb],
                                start=(cb == 0),
                                stop=(cb == CB - 1),
                            )
                    dst = qT if qk == 0 else kT
                    nc.scalar.copy(
                        out=dst[:, :, TC * t : TC * (t + 1)],
                        in_=p2.rearrange("p (j f) -> p j f", j=JQK),
                    )
                # v in natural layout, one 128-token block at a time
                for tb in range(TC // 128):
                    tb_g = t * (TC // 128) + tb
                    pv = ps_med.tile([128, CD], FP32, tag="med")
                    for cb in range(CB):
                        nc.tensor.matmul(
                            pv,
                            xts[cb][:, 128 * tb : 128 * (tb + 1)],
                            w_sb[:, cb, 2 * CD : 3 * CD],
                            start=(cb == 0),
                            stop=(cb == CB - 1),
                        )
                    nc.scalar.copy(
                        out=vS[:, tb_g, :, 0:D],
                        in_=pv.rearrange("p (h d) -> p h d", d=D),
                    )

            # ================= phase 2: attention =================
            # Softmax tails (reciprocal -> ones-matmul broadcast -> divide)
            # are deferred: each (p, ic)'s tail is emitted after the next
            # iteration's first two QK groups, so by the time the PE
            # reaches the K=1 broadcast matmul its DVE-produced reciprocal
            # is long done and the PE never idles (keeps HAM warm).
            def emit_tail(p, ic, av):
                for h2 in range(2):
                    recip = small_pool.tile([65, ICH], FP16, tag="recip")
                    with nc.allow_low_precision("softmax recip fp16"):
                        nc.vector.reciprocal(
                            recip[64:65, :], av[h2][D : D + 1, :]
                        )
                    bc = ps_med.tile([64, ICH], FP32, tag="med", name="bc")
                    nc.tensor.matmul(
                        bc,
                        ones_sb[64:65, :],
                        recip[64:65, :],
                        start=True,
                        stop=True,
                        tile_position=(64, 0),
                    )
                    bc_sb = small_pool.tile([64, ICH], FP16, tag="bc")
                    nc.vector.tensor_copy(out=bc_sb, in_=bc)
                    if h2 == 0:
                        nc.vector.tensor_tensor(
                            attT[0:64, p, ICH * ic : ICH * (ic + 1)],
                            av[h2][0:D, :],
                            bc_sb,
                            ALU.mult,
                        )
                    else:
                        tmp = small_pool.tile([64, ICH], FP16, tag="tmp")
                        nc.vector.tensor_tensor(
                            tmp, av[h2][0:D, :], bc_sb, ALU.mult
                        )
                        # partition shift 0-63 -> 64-127 via DMA
                        nc.sync.dma_start(
                            attT[64:128, p, ICH * ic : ICH * (ic + 1)], tmp
                        )

            pending = None
            for p in range(NPAIR):
                for ic in range(NI):
                    J = S * (ic + 1)      # kept key blocks for this chunk
                    n_grp = (J + 1) // 2
                    av = [
                        ps_med.tile([D + 1, ICH], FP32, tag="med", name=f"av{h2}")
                        for h2 in range(2)
                    ]

                    def qk_group(g, p=p, ic=ic, J=J):
                        """scores^T + exp + causal mask for 2 j-blocks,
                        both heads of the pair. Returns ew tiles."""
                        jbs = [jb for jb in (2 * g, 2 * g + 1) if jb < J]
                        ews = []
                        for h2 in range(2):
                            po = 64 * h2
                            sc = ps_big.tile(
                                [128, len(jbs) * ICH], FP32, tag="big"
                            )
                            for i_s, jb in enumerate(jbs):
                                nc.tensor.matmul(
                                    sc[:, ICH * i_s : ICH * (i_s + 1)],
                                    kT[po : po + 64, p, 128 * jb : 128 * (jb + 1)],
                                    qT[po : po + 64, p, ICH * ic : ICH * (ic + 1)],
                                    start=True,
                                    stop=True,
                                    tile_position=(po, 0),
                                )
                            ew = ew_pool.tile(
                                [128, len(jbs), ICH], FP16, tag="ew"
                            )
                            nc.scalar.activation(
                                ew.rearrange("p a b -> p (a b)"),
                                sc,
                                AF.Exp,
                                scale=softmax_scale,
                            )
                            for i_s, jb in enumerate(jbs):
                                s = jb - S * ic
                                if s >= 0:   # diagonal: zero where j > i
                                    nc.gpsimd.affine_select(
                                        out=ew[:, i_s],
                                        in_=ew[:, i_s],
                                        compare_op=ALU.is_ge,
                                        fill=0.0,
                                        base=-128 * s,
                                        pattern=[[1, ICH]],
                                        channel_multiplier=-1,
                                    )
                            ews.append((ew, jbs))
                        return ews

                    def av_group(ews, p=p, J=J):
                        for h2 in range(2):
                            ew, jbs = ews[h2]
                            for i_s, jb in enumerate(jbs):
                                nc.tensor.matmul(
                                    av[h2],
                                    vS[:, jb, 2 * p + h2, :],
                                    ew[:, i_s],
                                    start=(jb == 0),
                                    stop=(jb == J - 1),
                                )

                    # 1-group software pipeline: QK(g+1) before AV(g);
                    # previous iteration's tail goes after the second QK
                    # group so its broadcast matmul never stalls the PE.
                    prev = qk_group(0)
                    for g in range(1, n_grp):
                        cur = qk_group(g)
                        if pending is not None:
                            emit_tail(*pending)
                            pending = None
                        av_group(prev)
                        prev = cur
                    if pending is not None:
                        emit_tail(*pending)
                        pending = None
                    av_group(prev)
                    pending = (p, ic, av)
            emit_tail(*pending)

            # ================= phase 3: projection =================
            for tb in range(NTB):
                for nck in range(C // 512):
                    py = ps_med.tile([128, 512], FP32, tag="med")
                    for p in range(NPAIR):
                        nc.tensor.matmul(
                            py,
                            attT[:, p, 128 * tb : 128 * (tb + 1)],
                            wp_sb[:, p, 512 * nck : 512 * (nck + 1)],
                            start=(p == 0),
                            stop=(p == NPAIR - 1),
                        )
                    y_sb = ysb_pool.tile([128, 512], FP16, tag="ysb")
                    nc.scalar.copy(out=y_sb, in_=py)
                    nc.sync.dma_start(
                        y[128 * tb : 128 * (tb + 1), 512 * nck : 512 * (nck + 1)],
                        y_sb,
                    )
    nc.compile()
    return nc


def make_in_maps(x, W_qkv, W_proj, n_cores=N_CORES):
    """Host-side sharding: per-core fp16 inputs."""
    Bv, T, C = x.shape
    groups = n_cores // Bv
    CD = C // groups
    in_maps = []
    xT_b = [np.ascontiguousarray(x[b].T).astype(np.float16) for b in range(Bv)]
    for m in range(n_cores):
        b, g = m // groups, m % groups
        cols = slice(CD * g, CD * (g + 1))
        wqkv_dev = np.ascontiguousarray(
            np.concatenate(
                [W_qkv[:, 0:C][:, cols], W_qkv[:, C : 2 * C][:, cols],
                 W_qkv[:, 2 * C : 3 * C][:, cols]],
                axis=1,
            ).astype(np.float16)
        )
        wp_dev = np.ascontiguousarray(W_proj[cols, :]).astype(np.float16)
        in_maps.append({"xT": xT_b[b], "wqkv": wqkv_dev, "wp": wp_dev})
    return in_maps


_cache = {}


def _get_nc():
    if "nc" not in _cache:
        _cache["nc"] = build_nc()
    return _cache["nc"]


def run(x, W_qkv, W_proj, trace=False):
    """Run on hardware; returns (y_full, BassKernelResults)."""
    from concourse.bass_utils import run_bass_kernel_spmd

    nc = _get_nc()
    in_maps = make_in_maps(x, W_qkv, W_proj)
    res = run_bass_kernel_spmd(
        nc, in_maps, core_ids=list(range(N_CORES)), trace=trace
    )
    Bv, T, C = x.shape
    groups = N_CORES // Bv
    y_full = np.zeros((Bv, T, C), dtype=np.float32)
    for m in range(N_CORES):
        y_full[m // groups] += res.results[m]["y"].astype(np.float32)
    return y_full, res


def kernel(x, W_qkv, W_proj):
    y, _ = run(
        np.asarray(x, dtype=np.float32),
        np.asarray(W_qkv, dtype=np.float32),
        np.asarray(W_proj, dtype=np.float32),
    )
    return y


# revision 19
# speedup vs baseline: 1.0284x; 1.0117x over previous
"""Causal self-attention Trainium2 kernel (8 NeuronCores, SPMD).

Problem (hardcoded): B=2, T=2048, C=1024, H=16 heads, d=64.
  qkv = x @ W_qkv ; per-head causal softmax attention ; out @ W_proj.

Sharding: core m (0..7) handles batch b = m//4 and head group g = m%4
(heads 4g..4g+3). Each core computes q/k/v for its 4 heads (256 of the
3072 W_qkv columns), full TxT causal attention for those heads, and a
partial projection y_m = att_m @ W_proj[256g:256g+256, :].  The host
sums the 4 partials per batch (row-split tensor parallel reduce).

Device kernel layout notes (scores kept TRANSPOSED: [key j, query i]):
  - x is fed pre-transposed per batch: xT [C, T] (fp16).
  - qkv^T computed as matmul(lhsT=W block, rhs=xT block): q^T/k^T land
    in [head-ch, T] layout, exactly what QK^T needs (contract over d).
  - v is computed in natural [T, ch] layout (lhsT=xT block, rhs=Wv) and
    stored with an extra ones column per head, so the AV matmul also
    yields the softmax denominators as output row 64 (M = d+1 = 65).
  - scores^T tiles [128 j, 512 i]: only j-blocks <= diagonal are
    computed (causal skip ~2x FLOPs); diagonal tiles are masked AFTER
    exp by zeroing with gpsimd.affine_select (keep iff j <= i).
  - softmax divide: reciprocal of the sums row, broadcast across 64
    partitions with a K=1 ones-matmul, multiplied in while copying out
    of PSUM. Head 1 of each pair is partition-shifted 0-63 -> 64-127
    with an SBUF->SBUF DMA so att^T tiles [128 = 2 heads x 64, T] feed
    the projection matmul directly as lhsT.
"""

import numpy as np

import concourse.bass as bass
import concourse.mybir as mybir
import concourse.tile as tile
from concourse import bacc

FP32 = mybir.dt.float32
FP32R = mybir.dt.float32r
FP16 = mybir.dt.float16
AF = mybir.ActivationFunctionType
ALU = mybir.AluOpType

B, T_FULL, C_FULL, H_FULL, D_HEAD = 2, 2048, 1024, 16, 64
N_CORES = 8


def build_nc(T=T_FULL, C=C_FULL, HD=4, D=D_HEAD, n_cores=N_CORES):
    """Build the per-core Bass program. HD = heads per core."""
    CD = HD * D              # device head channels (256)
    CB = C // 128            # contraction blocks over x/W channels
    ICH = 512                # query-chunk width
    NI = T // ICH
    S = ICH // 128           # j-blocks per query chunk on the diagonal
    TC = 512                 # token chunk in qkv phase
    NTC = T // TC
    NTB = T // 128           # 128-token blocks (= key blocks)
    NPAIR = HD // 2
    JQK = CD // 128          # q (and k) 128-wide column blocks
    assert JQK == NPAIR and T % ICH == 0 and C % 512 == 0
    softmax_scale = 1.0 / float(np.sqrt(D))

    nc = bacc.Bacc(
        "TRN2", target_bir_lowering=False, debug=False, num_devices=n_cores
    )
    xT = nc.dram_tensor("xT", [C, T], FP16, kind="ExternalInput").ap()
    wqkv = nc.dram_tensor("wqkv", [C, 3 * CD], FP16, kind="ExternalInput").ap()
    wp = nc.dram_tensor("wp", [CD, C], FP16, kind="ExternalInput").ap()
    y = nc.dram_tensor("y", [T, C], FP16, kind="ExternalOutput").ap()

    with tile.TileContext(nc) as tc:
        with (
            tc.tile_pool(name="consts", bufs=1) as consts,
            tc.tile_pool(name="xt", bufs=2 * CB) as xt_pool,
            tc.tile_pool(name="ew", bufs=4) as ew_pool,
            tc.tile_pool(name="small", bufs=4) as small_pool,
            tc.tile_pool(name="ysb", bufs=4) as ysb_pool,
            tc.tile_pool(name="psb", bufs=2, space="PSUM") as ps_big,
            tc.tile_pool(name="psm", bufs=4, space="PSUM") as ps_med,
        ):
            # ---- resident tensors ----
            w_sb = consts.tile([128, CB, 3 * CD], FP16)
            nc.sync.dma_start(w_sb, wqkv.rearrange("(po pi) f -> pi po f", pi=128))
            wp_sb = consts.tile([128, CD // 128, C], FP16)
            nc.sync.dma_start(wp_sb, wp.rearrange("(po pi) f -> pi po f", pi=128))
            qT = consts.tile([128, NPAIR, T], FP16)
            kT = consts.tile([128, NPAIR, T], FP16)
            vS = consts.tile([128, NTB, HD, D + 1], FP16)
            nc.vector.memset(vS[:, :, :, D : D + 1], 1.0)
            attT = consts.tile([128, NPAIR, T], FP16)
            ones_sb = consts.tile([65, 64], FP16)
            nc.vector.memset(ones_sb[64:65, :], 1.0)

            # ================= phase 1: qkv =================
            for t in range(NTC):
                xts = []
                for cb in range(CB):
                    xt_t = xt_pool.tile([128, TC], FP16, tag="xt")
                    nc.sync.dma_start(
                        xt_t, xT[128 * cb : 128 * (cb + 1), TC * t : TC * (t + 1)]
                    )
                    xts.append(xt_t)
                # q^T / k^T: one [128, 2*TC] psum holds both column blocks
                for qk in range(2):       # 0 -> q, 1 -> k
                    p2 = ps_big.tile([128, JQK * TC], FP32, tag="big")
                    for jb in range(JQK):
                        co = CD * qk + 128 * jb
                        for cb in range(CB):
                            nc.tensor.matmul(
                                p2[:, TC * jb : TC * (jb + 1)],
                                w_sb[:, cb, co : co + 128],
                                xts[cb],
                                start=(cb == 0),
                                stop=(cb == CB - 1),
                            )
                    dst = qT if qk == 0 else kT
                    nc.scalar.copy(
                        out=dst[:, :, TC * t : TC * (t + 1)],
                        in_=p2.rearrange("p (j f) -> p j f", j=JQK),
                    )
                # v in natural layout, one 128-token block at a time
                for tb in range(TC // 128):
                    tb_g = t * (TC // 128) + tb
                    pv = ps_med.tile([128, CD], FP32, tag="med")
                    for cb in range(CB):
                        nc.tensor.matmul(
                            pv,
                            xts[cb][:, 128 * tb : 128 * (tb + 1)],
                            w_sb[:, cb, 2 * CD : 3 * CD],
                            start=(cb == 0),
                            stop=(cb == CB - 1),
                        )
                    nc.scalar.copy(
                        out=vS[:, tb_g, :, 0:D],
                        in_=pv.rearrange("p (h d) -> p h d", d=D),
                    )

            # ================= phase 2: attention =================
            # Softmax tails (reciprocal -> ones-matmul broadcast -> divide)
            # are deferred: each (p, ic)'s tail is emitted after the next
            # iteration's first two QK groups, so by the time the PE
            # reaches the K=1 broadcast matmul its DVE-produced reciprocal
            # is long done and the PE never idles (keeps HAM warm).
            def emit_tail(p, ic, av):
                # Broadcast the RAW sums row with the K=1 ones-matmul; the
                # only thing the PE waits on is a 0.4us DVE cast of the
                # sums row to fp16. The slow reciprocal then runs on the
                # broadcast result entirely off the PE critical path.
                sums16 = []
                for h2 in range(2):
                    s16 = small_pool.tile([65, ICH], FP16, tag="recip")
                    with nc.allow_low_precision("softmax sums fp16"):
                        nc.vector.tensor_copy(
                            out=s16[64:65, :], in_=av[h2][D : D + 1, :]
                        )
                    sums16.append(s16)
                bcs = []
                for h2 in range(2):
                    bc = ps_med.tile([64, ICH], FP32, tag="med", name="bc")
                    nc.tensor.matmul(
                        bc,
                        ones_sb[64:65, :],
                        sums16[h2][64:65, :],
                        start=True,
                        stop=True,
                        tile_position=(64, 0),
                    )
                    bcs.append(bc)
                for h2 in range(2):
                    bc_sb = small_pool.tile([64, ICH], FP16, tag="bc")
                    with nc.allow_low_precision("softmax recip fp16"):
                        nc.vector.reciprocal(bc_sb, bcs[h2])
                    if h2 == 0:
                        nc.vector.tensor_tensor(
                            attT[0:64, p, ICH * ic : ICH * (ic + 1)],
                            av[h2][0:D, :],
                            bc_sb,
                            ALU.mult,
                        )
                    else:
                        tmp = small_pool.tile([64, ICH], FP16, tag="tmp")
                        nc.vector.tensor_tensor(
                            tmp, av[h2][0:D, :], bc_sb, ALU.mult
                        )
                        # partition shift 0-63 -> 64-127 via DMA
                        nc.sync.dma_start(
                            attT[64:128, p, ICH * ic : ICH * (ic + 1)], tmp
                        )

            pending = None
            for p in range(NPAIR):
                for ic in range(NI):
                    J = S * (ic + 1)      # kept key blocks for this chunk
                    n_grp = (J + 1) // 2
                    av = [
                        ps_med.tile([D + 1, ICH], FP32, tag="med", name=f"av{h2}")
                        for h2 in range(2)
                    ]

                    def qk_group(g, p=p, ic=ic, J=J):
                        """scores^T + exp + causal mask for 2 j-blocks,
                        both heads of the pair. Returns ew tiles."""
                        jbs = [jb for jb in (2 * g, 2 * g + 1) if jb < J]
                        ews = []
                        for h2 in range(2):
                            po = 64 * h2
                            sc = ps_big.tile(
                                [128, len(jbs) * ICH], FP32, tag="big"
                            )
                            for i_s, jb in enumerate(jbs):
                                nc.tensor.matmul(
                                    sc[:, ICH * i_s : ICH * (i_s + 1)],
                                    kT[po : po + 64, p, 128 * jb : 128 * (jb + 1)],
                                    qT[po : po + 64, p, ICH * ic : ICH * (ic + 1)],
                                    start=True,
                                    stop=True,
                                    tile_position=(po, 0),
                                )
                            ew = ew_pool.tile(
                                [128, len(jbs), ICH], FP16, tag="ew"
                            )
                            nc.scalar.activation(
                                ew.rearrange("p a b -> p (a b)"),
                                sc,
                                AF.Exp,
                                scale=softmax_scale,
                            )
                            for i_s, jb in enumerate(jbs):
                                s = jb - S * ic
                                if s >= 0:   # diagonal: zero where j > i
                                    nc.gpsimd.affine_select(
                                        out=ew[:, i_s],
                                        in_=ew[:, i_s],
                                        compare_op=ALU.is_ge,
                                        fill=0.0,
                                        base=-128 * s,
                                        pattern=[[1, ICH]],
                                        channel_multiplier=-1,
                                    )
                            ews.append((ew, jbs))
                        return ews

                    def av_group(ews, p=p, J=J):
                        for h2 in range(2):
                            ew, jbs = ews[h2]
                            for i_s, jb in enumerate(jbs):
                                nc.tensor.matmul(
                                    av[h2],
                                    vS[:, jb, 2 * p + h2, :],
                                    ew[:, i_s],
                                    start=(jb == 0),
                                    stop=(jb == J - 1),
                                )

                    # 1-group software pipeline: QK(g+1) before AV(g);
                    # previous iteration's tail goes after the second QK
                    # group so its broadcast matmul never stalls the PE.
                    prev = qk_group(0)
                    for g in range(1, n_grp):
                        cur = qk_group(g)
                        if pending is not None:
                            emit_tail(*pending)
                            pending = None
                        av_group(prev)
                        prev = cur
                    if pending is not None:
                        emit_tail(*pending)
                        pending = None
                    av_group(prev)
                    pending = (p, ic, av)
            emit_tail(*pending)

            # ================= phase 3: projection =================
            for tb in range(NTB):
                for nck in range(C // 512):
                    py = ps_med.tile([128, 512], FP32, tag="med")
                    for p in range(NPAIR):
                        nc.tensor.matmul(
                            py,
                            attT[:, p, 128 * tb : 128 * (tb + 1)],
                            wp_sb[:, p, 512 * nck : 512 * (nck + 1)],
                            start=(p == 0),
                            stop=(p == NPAIR - 1),
                        )
                    y_sb = ysb_pool.tile([128, 512], FP16, tag="ysb")
                    nc.scalar.copy(out=y_sb, in_=py)
                    nc.sync.dma_start(
                        y[128 * tb : 128 * (tb + 1), 512 * nck : 512 * (nck + 1)],
                        y_sb,
                    )
    nc.compile()
    return nc


def make_in_maps(x, W_qkv, W_proj, n_cores=N_CORES):
    """Host-side sharding: per-core fp16 inputs."""
    Bv, T, C = x.shape
    groups = n_cores // Bv
    CD = C // groups
    in_maps = []
    xT_b = [np.ascontiguousarray(x[b].T).astype(np.float16) for b in range(Bv)]
    for m in range(n_cores):
        b, g = m // groups, m % groups
        cols = slice(CD * g, CD * (g + 1))
        wqkv_dev = np.ascontiguousarray(
            np.concatenate(
                [W_qkv[:, 0:C][:, cols], W_qkv[:, C : 2 * C][:, cols],
                 W_qkv[:, 2 * C : 3 * C][:, cols]],
                axis=1,
            ).astype(np.float16)
        )
        wp_dev = np.ascontiguousarray(W_proj[cols, :]).astype(np.float16)
        in_maps.append({"xT": xT_b[b], "wqkv": wqkv_dev, "wp": wp_dev})
    return in_maps


_cache = {}


def _get_nc():
    if "nc" not in _cache:
        _cache["nc"] = build_nc()
    return _cache["nc"]


def run(x, W_qkv, W_proj, trace=False):
    """Run on hardware; returns (y_full, BassKernelResults)."""
    from concourse.bass_utils import run_bass_kernel_spmd

    nc = _get_nc()
    in_maps = make_in_maps(x, W_qkv, W_proj)
    res = run_bass_kernel_spmd(
        nc, in_maps, core_ids=list(range(N_CORES)), trace=trace
    )
    Bv, T, C = x.shape
    groups = N_CORES // Bv
    y_full = np.zeros((Bv, T, C), dtype=np.float32)
    for m in range(N_CORES):
        y_full[m // groups] += res.results[m]["y"].astype(np.float32)
    return y_full, res


def kernel(x, W_qkv, W_proj):
    y, _ = run(
        np.asarray(x, dtype=np.float32),
        np.asarray(W_qkv, dtype=np.float32),
        np.asarray(W_proj, dtype=np.float32),
    )
    return y


# revision 20
# speedup vs baseline: 1.0986x; 1.0683x over previous
"""Causal self-attention Trainium2 kernel (8 NeuronCores, SPMD).

Problem (hardcoded): B=2, T=2048, C=1024, H=16 heads, d=64.
  qkv = x @ W_qkv ; per-head causal softmax attention ; out @ W_proj.

Sharding: core m (0..7) handles batch b = m//4 and head group g = m%4
(heads 4g..4g+3). Each core computes q/k/v for its 4 heads (256 of the
3072 W_qkv columns), full TxT causal attention for those heads, and a
partial projection y_m = att_m @ W_proj[256g:256g+256, :].  The host
sums the 4 partials per batch (row-split tensor parallel reduce).

Device kernel layout notes (scores kept TRANSPOSED: [key j, query i]):
  - x is fed pre-transposed per batch: xT [C, T] (fp16).
  - qkv^T computed as matmul(lhsT=W block, rhs=xT block): q^T/k^T land
    in [head-ch, T] layout, exactly what QK^T needs (contract over d).
  - v is computed in natural [T, ch] layout (lhsT=xT block, rhs=Wv) and
    stored with an extra ones column per head, so the AV matmul also
    yields the softmax denominators as output row 64 (M = d+1 = 65).
  - scores^T tiles [128 j, 512 i]: only j-blocks <= diagonal are
    computed (causal skip ~2x FLOPs); diagonal tiles are masked AFTER
    exp by zeroing with gpsimd.affine_select (keep iff j <= i).
  - softmax divide: reciprocal of the sums row, broadcast across 64
    partitions with a K=1 ones-matmul, multiplied in while copying out
    of PSUM. Head 1 of each pair is partition-shifted 0-63 -> 64-127
    with an SBUF->SBUF DMA so att^T tiles [128 = 2 heads x 64, T] feed
    the projection matmul directly as lhsT.
"""

import numpy as np

import concourse.bass as bass
import concourse.mybir as mybir
import concourse.tile as tile
from concourse import bacc

FP32 = mybir.dt.float32
FP32R = mybir.dt.float32r
FP16 = mybir.dt.float16
AF = mybir.ActivationFunctionType
ALU = mybir.AluOpType

B, T_FULL, C_FULL, H_FULL, D_HEAD = 2, 2048, 1024, 16, 64
N_CORES = 8


def build_nc(T=T_FULL, C=C_FULL, HD=4, D=D_HEAD, n_cores=N_CORES):
    """Build the per-core Bass program. HD = heads per core."""
    CD = HD * D              # device head channels (256)
    CB = C // 128            # contraction blocks over x/W channels
    ICH = 512                # query-chunk width
    NI = T // ICH
    S = ICH // 128           # j-blocks per query chunk on the diagonal
    TC = 512                 # token chunk in qkv phase
    NTC = T // TC
    NTB = T // 128           # 128-token blocks (= key blocks)
    NPAIR = HD // 2
    JQK = CD // 128          # q (and k) 128-wide column blocks
    assert JQK == NPAIR and T % ICH == 0 and C % 512 == 0
    softmax_scale = 1.0 / float(np.sqrt(D))

    nc = bacc.Bacc(
        "TRN2", target_bir_lowering=False, debug=False, num_devices=n_cores
    )
    xT = nc.dram_tensor("xT", [C, T], FP16, kind="ExternalInput").ap()
    wqkv = nc.dram_tensor("wqkv", [C, 3 * CD], FP16, kind="ExternalInput").ap()
    wp = nc.dram_tensor("wp", [CD, C], FP16, kind="ExternalInput").ap()
    y = nc.dram_tensor("y", [T, C], FP16, kind="ExternalOutput").ap()

    with tile.TileContext(nc) as tc:
        with (
            tc.tile_pool(name="consts", bufs=1) as consts,
            tc.tile_pool(name="xt", bufs=2 * CB) as xt_pool,
            tc.tile_pool(name="ew", bufs=4) as ew_pool,
            tc.tile_pool(name="small", bufs=4) as small_pool,
            tc.tile_pool(name="ysb", bufs=4) as ysb_pool,
            tc.tile_pool(name="psb", bufs=2, space="PSUM") as ps_big,
            tc.tile_pool(name="psm", bufs=4, space="PSUM") as ps_med,
        ):
            # ---- resident tensors ----
            w_sb = consts.tile([128, CB, 3 * CD], FP16)
            nc.sync.dma_start(w_sb, wqkv.rearrange("(po pi) f -> pi po f", pi=128))
            wp_sb = consts.tile([128, CD // 128, C], FP16)
            nc.sync.dma_start(wp_sb, wp.rearrange("(po pi) f -> pi po f", pi=128))
            qT = consts.tile([128, NPAIR, T], FP16)
            kT = consts.tile([128, NPAIR, T], FP16)
            vS = consts.tile([128, NTB, HD, D + 1], FP16)
            nc.vector.memset(vS[:, :, :, D : D + 1], 1.0)
            attT = consts.tile([128, NPAIR, T], FP16)
            ones_sb = consts.tile([65, 64], FP16)
            nc.vector.memset(ones_sb[64:65, :], 1.0)

            # ================= phase 1: qkv =================
            for t in range(NTC):
                xts = []
                for cb in range(CB):
                    xt_t = xt_pool.tile([128, TC], FP16, tag="xt")
                    nc.sync.dma_start(
                        xt_t, xT[128 * cb : 128 * (cb + 1), TC * t : TC * (t + 1)]
                    )
                    xts.append(xt_t)
                # q^T / k^T: one [128, 2*TC] psum holds both column blocks
                for qk in range(2):       # 0 -> q, 1 -> k
                    p2 = ps_big.tile([128, JQK * TC], FP32, tag="big")
                    for jb in range(JQK):
                        co = CD * qk + 128 * jb
                        for cb in range(CB):
                            nc.tensor.matmul(
                                p2[:, TC * jb : TC * (jb + 1)],
                                w_sb[:, cb, co : co + 128],
                                xts[cb],
                                start=(cb == 0),
                                stop=(cb == CB - 1),
                            )
                    dst = qT if qk == 0 else kT
                    nc.scalar.copy(
                        out=dst[:, :, TC * t : TC * (t + 1)],
                        in_=p2.rearrange("p (j f) -> p j f", j=JQK),
                    )
                # v in natural layout, one 128-token block at a time
                for tb in range(TC // 128):
                    tb_g = t * (TC // 128) + tb
                    pv = ps_med.tile([128, CD], FP32, tag="med")
                    for cb in range(CB):
                        nc.tensor.matmul(
                            pv,
                            xts[cb][:, 128 * tb : 128 * (tb + 1)],
                            w_sb[:, cb, 2 * CD : 3 * CD],
                            start=(cb == 0),
                            stop=(cb == CB - 1),
                        )
                    nc.scalar.copy(
                        out=vS[:, tb_g, :, 0:D],
                        in_=pv.rearrange("p (h d) -> p h d", d=D),
                    )

            # ================= phase 2: attention =================
            # Softmax tails (reciprocal -> ones-matmul broadcast -> divide)
            # are deferred: each (p, ic)'s tail is emitted after the next
            # iteration's first two QK groups, so by the time the PE
            # reaches the K=1 broadcast matmul its DVE-produced reciprocal
            # is long done and the PE never idles (keeps HAM warm).
            def emit_tail(p, ic, av):
                # Evacuate PSUM fast: copy the raw av data + sums rows out
                # first (releases the av banks in ~1us so the next chunk's
                # AV matmuls aren't slot-starved), broadcast the raw sums
                # with the K=1 ones-matmul, then run the slow reciprocal +
                # in-place normalize entirely off the PE critical path.
                sums16 = []
                tmp1 = None
                for h2 in range(2):
                    if h2 == 0:
                        with nc.allow_low_precision("raw attT fp16"):
                            nc.vector.tensor_copy(
                                out=attT[0:64, p, ICH * ic : ICH * (ic + 1)],
                                in_=av[0][0:D, :],
                            )
                    else:
                        tmp1 = small_pool.tile([64, ICH], FP16, tag="tmp")
                        with nc.allow_low_precision("raw attT fp16"):
                            nc.vector.tensor_copy(out=tmp1, in_=av[1][0:D, :])
                    s16 = small_pool.tile([65, ICH], FP16, tag="recip")
                    with nc.allow_low_precision("softmax sums fp16"):
                        nc.vector.tensor_copy(
                            out=s16[64:65, :], in_=av[h2][D : D + 1, :]
                        )
                    sums16.append(s16)
                bcs = []
                for h2 in range(2):
                    bc = ps_med.tile([64, ICH], FP32, tag="med", name="bc")
                    nc.tensor.matmul(
                        bc,
                        ones_sb[64:65, :],
                        sums16[h2][64:65, :],
                        start=True,
                        stop=True,
                        tile_position=(64, 0),
                    )
                    bcs.append(bc)
                for h2 in range(2):
                    bc_sb = small_pool.tile([64, ICH], FP16, tag="bc")
                    with nc.allow_low_precision("softmax recip fp16"):
                        nc.vector.reciprocal(bc_sb, bcs[h2])
                    if h2 == 0:
                        dst = attT[0:64, p, ICH * ic : ICH * (ic + 1)]
                        nc.vector.tensor_tensor(dst, dst, bc_sb, ALU.mult)
                    else:
                        nc.vector.tensor_tensor(tmp1, tmp1, bc_sb, ALU.mult)
                        # partition shift 0-63 -> 64-127 via DMA
                        nc.sync.dma_start(
                            attT[64:128, p, ICH * ic : ICH * (ic + 1)], tmp1
                        )

            pending = None
            for p in range(NPAIR):
                for ic in range(NI):
                    J = S * (ic + 1)      # kept key blocks for this chunk
                    n_grp = (J + 1) // 2
                    av = [
                        ps_med.tile([D + 1, ICH], FP32, tag="med", name=f"av{h2}")
                        for h2 in range(2)
                    ]

                    def qk_group(g, p=p, ic=ic, J=J):
                        """scores^T + exp + causal mask for 2 j-blocks,
                        both heads of the pair. Returns ew tiles."""
                        jbs = [jb for jb in (2 * g, 2 * g + 1) if jb < J]
                        ews = []
                        for h2 in range(2):
                            po = 64 * h2
                            sc = ps_big.tile(
                                [128, len(jbs) * ICH], FP32, tag="big"
                            )
                            for i_s, jb in enumerate(jbs):
                                nc.tensor.matmul(
                                    sc[:, ICH * i_s : ICH * (i_s + 1)],
                                    kT[po : po + 64, p, 128 * jb : 128 * (jb + 1)],
                                    qT[po : po + 64, p, ICH * ic : ICH * (ic + 1)],
                                    start=True,
                                    stop=True,
                                    tile_position=(po, 0),
                                )
                            ew = ew_pool.tile(
                                [128, len(jbs), ICH], FP16, tag="ew"
                            )
                            nc.scalar.activation(
                                ew.rearrange("p a b -> p (a b)"),
                                sc,
                                AF.Exp,
                                scale=softmax_scale,
                            )
                            for i_s, jb in enumerate(jbs):
                                s = jb - S * ic
                                if s >= 0:   # diagonal: zero where j > i
                                    nc.gpsimd.affine_select(
                                        out=ew[:, i_s],
                                        in_=ew[:, i_s],
                                        compare_op=ALU.is_ge,
                                        fill=0.0,
                                        base=-128 * s,
                                        pattern=[[1, ICH]],
                                        channel_multiplier=-1,
                                    )
                            ews.append((ew, jbs))
                        return ews

                    def av_group(ews, p=p, J=J):
                        for h2 in range(2):
                            ew, jbs = ews[h2]
                            for i_s, jb in enumerate(jbs):
                                nc.tensor.matmul(
                                    av[h2],
                                    vS[:, jb, 2 * p + h2, :],
                                    ew[:, i_s],
                                    start=(jb == 0),
                                    stop=(jb == J - 1),
                                )

                    # 1-group software pipeline: QK(g+1) before AV(g);
                    # previous iteration's tail goes after the second QK
                    # group so its broadcast matmul never stalls the PE.
                    prev = qk_group(0)
                    for g in range(1, n_grp):
                        cur = qk_group(g)
                        if pending is not None:
                            emit_tail(*pending)
                            pending = None
                        av_group(prev)
                        prev = cur
                    if pending is not None:
                        emit_tail(*pending)
                        pending = None
                    av_group(prev)
                    pending = (p, ic, av)
            emit_tail(*pending)

            # ================= phase 3: projection =================
            for tb in range(NTB):
                for nck in range(C // 512):
                    py = ps_med.tile([128, 512], FP32, tag="med")
                    for p in range(NPAIR):
                        nc.tensor.matmul(
                            py,
                            attT[:, p, 128 * tb : 128 * (tb + 1)],
                            wp_sb[:, p, 512 * nck : 512 * (nck + 1)],
                            start=(p == 0),
                            stop=(p == NPAIR - 1),
                        )
                    y_sb = ysb_pool.tile([128, 512], FP16, tag="ysb")
                    nc.scalar.copy(out=y_sb, in_=py)
                    nc.sync.dma_start(
                        y[128 * tb : 128 * (tb + 1), 512 * nck : 512 * (nck + 1)],
                        y_sb,
                    )
    nc.compile()
    return nc


def make_in_maps(x, W_qkv, W_proj, n_cores=N_CORES):
    """Host-side sharding: per-core fp16 inputs."""
    Bv, T, C = x.shape
    groups = n_cores // Bv
    CD = C // groups
    in_maps = []
    xT_b = [np.ascontiguousarray(x[b].T).astype(np.float16) for b in range(Bv)]
    for m in range(n_cores):
        b, g = m // groups, m % groups
        cols = slice(CD * g, CD * (g + 1))
        wqkv_dev = np.ascontiguousarray(
            np.concatenate(
                [W_qkv[:, 0:C][:, cols], W_qkv[:, C : 2 * C][:, cols],
                 W_qkv[:, 2 * C : 3 * C][:, cols]],
                axis=1,
            ).astype(np.float16)
        )
        wp_dev = np.ascontiguousarray(W_proj[cols, :]).astype(np.float16)
        in_maps.append({"xT": xT_b[b], "wqkv": wqkv_dev, "wp": wp_dev})
    return in_maps


_cache = {}


def _get_nc():
    if "nc" not in _cache:
        _cache["nc"] = build_nc()
    return _cache["nc"]


def run(x, W_qkv, W_proj, trace=False):
    """Run on hardware; returns (y_full, BassKernelResults)."""
    from concourse.bass_utils import run_bass_kernel_spmd

    nc = _get_nc()
    in_maps = make_in_maps(x, W_qkv, W_proj)
    res = run_bass_kernel_spmd(
        nc, in_maps, core_ids=list(range(N_CORES)), trace=trace
    )
    Bv, T, C = x.shape
    groups = N_CORES // Bv
    y_full = np.zeros((Bv, T, C), dtype=np.float32)
    for m in range(N_CORES):
        y_full[m // groups] += res.results[m]["y"].astype(np.float32)
    return y_full, res


def kernel(x, W_qkv, W_proj):
    y, _ = run(
        np.asarray(x, dtype=np.float32),
        np.asarray(W_qkv, dtype=np.float32),
        np.asarray(W_proj, dtype=np.float32),
    )
    return y
